# revision 1
# baseline (speedup 1.0000x reference)
"""Trainium2 Bass kernel for nn_DecodeBlock (RetNet-style decoder block), v2.

Sharding: data-parallel over batch (B=8) across 8 NeuronCores; no collectives.

Design (per core, vs the quadratic v1 baseline):
  - Chunked-recurrent retention (C=128): per chunk, intra-chunk scores
    [128,128] + cross-chunk contribution through a per-head [dk,dv] state
    accumulated in PSUM across chunks (global kappa^±n scaling keeps the
    recurrence a pure sum; exact, no approximation).
  - fp8-e4m3 DoubleRow matmuls (2 k-tiles per pass, 0.5 cyc/row) for the
    K_seq/gate/W_O/FFN gemms; q/k/V projections stay bf16 (precision).
  - Sequence-major normalization path: GroupNorm/RMSNorm stats as [128,8]
    narrow tiles (engine cost scales with free-size), ACT per-partition
    Rsqrt/scale application, residuals fused into gemm evacuations.
  - All transposes via the DMA XBAR (dma_start_transpose, bf16), not PE.
  - Output is produced sequence-major and DMA'd straight out.
"""

import numpy as np

import concourse.bass as bass
import concourse.mybir as mybir
import concourse.tile as tile
from concourse.bass_utils import run_bass_kernel_spmd

F32 = mybir.dt.float32
BF16 = mybir.dt.bfloat16
FP8 = mybir.dt.float8e4
AF = mybir.ActivationFunctionType
ALU = mybir.AluOpType
DRM = mybir.MatmulPerfMode.DoubleRow

E, H, B, S = 512, 8, 8, 1024
DH = E // H          # 64
P = 128
NF = E // P          # 4 feature tiles
NC = S // P          # 8 seq chunks

N_CORES = 8


def _kappas():
    k = 1.0 - np.exp(np.linspace(np.log(1.0 / 32.0), np.log(1.0 / 512.0), H))
    return k.astype(np.float64)


def _pair8(w):
    """[E, E] weight -> fp8 DR layout [128, 4*512]: col block j*512 = k-tile j
    (rows j*128..j*128+127)."""
    import ml_dtypes
    w = np.asarray(w, np.float32)
    return np.ascontiguousarray(
        w.reshape(NF, P, E).transpose(1, 0, 2).reshape(P, NF * E)
        .astype(ml_dtypes.float8_e4m3))


def _build_consts(inputs):
    import ml_dtypes
    bf16 = ml_dtypes.bfloat16
    kap = _kappas()
    n = np.arange(S, dtype=np.float64)
    kq = np.empty((E, S), np.float64)
    kk = np.empty((E, S), np.float64)
    for h in range(H):
        kq[h * DH:(h + 1) * DH, :] = (kap[h] ** n)[None, :]
        kk[h * DH:(h + 1) * DH, :] = (kap[h] ** (-n))[None, :]
    kks = np.empty((S, E), np.float64)   # seq-major kappa^-m, head-major cols
    for h in range(H):
        kks[:, h * DH:(h + 1) * DH] = (kap[h] ** (-n))[:, None]
    # causal keep n>=m, [128,128] repeated 4x along free
    cm = (np.arange(P)[None, :] >= np.arange(P)[:, None]).astype(np.float32)
    cmask4 = np.ascontiguousarray(np.tile(cm, (1, 4)).astype(bf16))

    ln1 = np.asarray(inputs["ln1_s"], np.float32)
    ln2 = np.asarray(inputs["ln2_s"], np.float32)

    def conc(w):
        return np.asarray(w, np.float32).transpose(1, 0, 2).reshape(E, E)

    wq1 = conc(inputs["wq1"]); wk1 = conc(inputs["wk1"]); wv1 = conc(inputs["wv1"])
    wq2 = conc(inputs["wq2"])
    wk2f = ln1[:, None] * conc(inputs["wk2"])   # fold ln1 into msr2 kv path
    wv2f = ln1[:, None] * conc(inputs["wv2"])
    fgf = ln2[:, None] * np.asarray(inputs["ffn_w_gate"], np.float32)
    flf = ln2[:, None] * np.asarray(inputs["ffn_w_lin"], np.float32)

    consts = {
        "kqm": np.ascontiguousarray(kq.astype(bf16)),
        "kkm": np.ascontiguousarray(kk.astype(bf16)),
        "kks": np.ascontiguousarray(kks.astype(bf16)),
        "cmask4": cmask4,
        "wqc1": np.ascontiguousarray(wq1.astype(bf16)),
        "wkc1": np.ascontiguousarray(wk1.astype(bf16)),
        "wvc1": np.ascontiguousarray(wv1.astype(bf16)),
        "wqc2": np.ascontiguousarray(wq2.astype(bf16)),
        "wkc2": np.ascontiguousarray(wk2f.astype(bf16)),
        "wvc2": np.ascontiguousarray(wv2f.astype(bf16)),
        "kp8_1": _pair8(wk1), "kp8_2": _pair8(wk2f),
        "wg8_1": _pair8(inputs["wg1"]), "wg8_2": _pair8(inputs["wg2"]),
        "wo8_1": _pair8(inputs["wo1"]), "wo8_2": _pair8(inputs["wo2"]),
        "fg8": _pair8(fgf), "fl8": _pair8(flf),
        "fo8": _pair8(inputs["ffn_w_out"]),
    }
    fl = _flags(inputs)
    if not fl[0]:
        consts["gsb1"] = np.ascontiguousarray(
            np.tile(np.asarray(inputs["gs1"], np.float32), (P, 1)))
        consts["gbb1"] = np.ascontiguousarray(
            np.tile(np.asarray(inputs["gb1"], np.float32), (P, 1)))
    if not fl[1]:
        consts["gsb2"] = np.ascontiguousarray(
            np.tile(np.asarray(inputs["gs2"], np.float32), (P, 1)))
        consts["gbb2"] = np.ascontiguousarray(
            np.tile(np.asarray(inputs["gb2"], np.float32), (P, 1)))
    if not fl[2]:
        consts["ln2C"] = np.ascontiguousarray(np.tile(ln2, (P, 1)))
    if not fl[3]:
        consts["ln3C"] = np.ascontiguousarray(
            np.tile(np.asarray(inputs["ln3_s"], np.float32), (P, 1)))
    return consts


def _flags(inputs):
    """(gn1 trivial, gn2 trivial, ln2 trivial, ln3 trivial)"""
    return (
        bool(np.allclose(inputs["gs1"], 1) and np.allclose(inputs["gb1"], 0)),
        bool(np.allclose(inputs["gs2"], 1) and np.allclose(inputs["gb2"], 0)),
        bool(np.allclose(inputs["ln2_s"], 1)),
        bool(np.allclose(inputs["ln3_s"], 1)),
    )


class _Prog:
    pass


def _strip_self_waits(nc):
    import concourse.mybir as mb
    for f in nc.m.functions:
        for blk in f.blocks:
            for inst in blk.instructions:
                si = getattr(inst, "sync_info", None)
                if si is None or not si.on_wait:
                    continue
                tname = type(inst).__name__
                if tname in ("InstDMACopy", "InstDrain", "InstEventSemaphore",
                             "InstTriggerDma", "InstDmaTransposeAnt"):
                    continue
                eng = getattr(inst, "engine", None)
                eng_name = getattr(eng, "name", str(eng))
                pref = {"PE": "PE_", "DVE": "DVE_", "Activation": "Activation_",
                        "Pool": "Pool_", "SP": "SP_"}.get(eng_name)
                if not pref:
                    continue
                kept = [w for w in si.on_wait if not str(w.ant_name).startswith(pref)]
                if len(kept) != len(si.on_wait):
                    si.on_wait = kept


_MAX_WAITS = 1
_WAIT_BUDGET = {"InstActivation": 1, "InstDrain": 0}


def _legalize_wait_counts(nc):
    import bass_rust
    import concourse.mybir as mb
    uid = [0]
    for f in nc.m.functions:
        for blk in f.blocks:
            insts = list(blk.instructions)
            out = []
            changed = False
            for inst in insts:
                si = getattr(inst, "sync_info", None)
                waits = list(si.on_wait) if si and si.on_wait else []
                plain = [w for w in waits if w.sync_type == "semaphore"]
                other = [w for w in waits if w.sync_type != "semaphore"]
                cap = _WAIT_BUDGET.get(type(inst).__name__, _MAX_WAITS)
                if len(plain) + len(other) > cap and len(plain) > 0:
                    budget = max(0, cap - len(other))
                    keep, excess = plain[:budget], plain[budget:]
                    while excess:
                        chunk, excess = excess[:1], excess[1:]
                        nop = bass_rust.InstNoOp(name=f"wnop-{uid[0]}", ins=[], outs=[])
                        uid[0] += 1
                        nop.engine = inst.engine
                        nop.sync_info = mb.SyncInfo(on_wait=chunk, on_update=[])
                        out.append(nop)
                    si.on_wait = other + keep
                    changed = True
                out.append(inst)
            if changed:
                blk.instructions = out


def _build_program(flags):
    nc = bass.Bass()
    pr = _Prog()
    pr.nc = nc
    d = {}
    d["x"] = nc.dram_tensor("x", [S, E], F32, kind="ExternalInput")
    d["obs"] = nc.dram_tensor("obs", [S, E], F32, kind="ExternalInput")
    for nm in ("wqc1", "wkc1", "wvc1", "wqc2", "wkc2", "wvc2"):
        d[nm] = nc.dram_tensor(nm, [E, E], BF16, kind="ExternalInput")
    for nm in ("kp8_1", "kp8_2", "wg8_1", "wg8_2", "wo8_1", "wo8_2",
               "fg8", "fl8", "fo8"):
        d[nm] = nc.dram_tensor(nm, [P, NF * E], FP8, kind="ExternalInput")
    d["kqm"] = nc.dram_tensor("kqm", [E, S], BF16, kind="ExternalInput")
    d["kkm"] = nc.dram_tensor("kkm", [E, S], BF16, kind="ExternalInput")
    d["kks"] = nc.dram_tensor("kks", [S, E], BF16, kind="ExternalInput")
    d["cmask4"] = nc.dram_tensor("cmask4", [P, 4 * P], BF16, kind="ExternalInput")
    gn1_triv, gn2_triv, ln2_triv, ln3_triv = flags
    if not gn1_triv:
        d["gsb1"] = nc.dram_tensor("gsb1", [P, E], F32, kind="ExternalInput")
        d["gbb1"] = nc.dram_tensor("gbb1", [P, E], F32, kind="ExternalInput")
    if not gn2_triv:
        d["gsb2"] = nc.dram_tensor("gsb2", [P, E], F32, kind="ExternalInput")
        d["gbb2"] = nc.dram_tensor("gbb2", [P, E], F32, kind="ExternalInput")
    if not ln2_triv:
        d["ln2C"] = nc.dram_tensor("ln2C", [P, E], F32, kind="ExternalInput")
    if not ln3_triv:
        d["ln3C"] = nc.dram_tensor("ln3C", [P, E], F32, kind="ExternalInput")
    out_h = nc.dram_tensor("out", [S, E], F32, kind="ExternalOutput")

    with tile.TileContext(nc) as tc:
        _emit(nc, tc, d, out_h, flags)
    _strip_self_waits(nc)
    _legalize_wait_counts(nc)
    return pr


def _ap3(t, off, d1, n1, d2, n2):
    """3D free AP over tile t: [partitions, (stride d1 x n1), (stride d2 x n2)]."""
    return bass.AP(tensor=t.tensor, offset=t.offset + off,
                   ap=[list(t.ap[0]), [d1, n1], [d2, n2]])


def _emit(nc, tc, d, out_h, flags):
    from contextlib import ExitStack
    gn1_triv, gn2_triv, ln2_triv, ln3_triv = flags
    import os
    tap = os.environ.get("KTAP", "")
    skips = set(os.environ.get("KSKIP", "").split(","))
    ctx = ExitStack()
    with ctx:
        p_c = ctx.enter_context(tc.tile_pool(name="const", bufs=1))
        p_w = ctx.enter_context(tc.tile_pool(name="w", bufs=2))
        p_w8 = ctx.enter_context(tc.tile_pool(name="w8", bufs=2))
        p_ld = ctx.enter_context(tc.tile_pool(name="ld", bufs=1))
        p_seq = ctx.enter_context(tc.tile_pool(name="seq", bufs=8))
        p_rot = ctx.enter_context(tc.tile_pool(name="rot", bufs=2))
        p_big = ctx.enter_context(tc.tile_pool(name="big", bufs=1))
        p_pair = ctx.enter_context(tc.tile_pool(name="pair", bufs=1))
        p_act = ctx.enter_context(tc.tile_pool(name="act", bufs=1))
        p_kv = ctx.enter_context(tc.tile_pool(name="kv", bufs=1))
        p_sc = ctx.enter_context(tc.tile_pool(name="scp", bufs=2))
        p_st = ctx.enter_context(tc.tile_pool(name="stp", bufs=1))
        p_sm = ctx.enter_context(tc.tile_pool(name="sm", bufs=4))
        p_res = ctx.enter_context(tc.tile_pool(name="res", bufs=2))
        pg = ctx.enter_context(tc.tile_pool(name="pg", bufs=2, space="PSUM"))
        psc = ctx.enter_context(tc.tile_pool(name="psc", bufs=2, space="PSUM"))
        pret = ctx.enter_context(tc.tile_pool(name="pret", bufs=2, space="PSUM"))
        pst = ctx.enter_context(tc.tile_pool(name="pst", bufs=1, space="PSUM"))
        pcro = ctx.enter_context(tc.tile_pool(name="pcro", bufs=1, space="PSUM"))

        # ---- consts ----
        cmask4 = p_c.tile([P, 4 * P], BF16)
        nc.sync.dma_start(out=cmask4, in_=d["cmask4"][:, :])
        eps_gn = p_c.tile([P, 1], F32)
        nc.vector.memset(eps_gn, 1e-5)
        eps_rms = p_c.tile([P, 1], F32)
        nc.vector.memset(eps_rms, 1e-6)
        gcons = {}
        for nm in ("gsb1", "gbb1", "gsb2", "gbb2", "ln2C", "ln3C"):
            if nm in d:
                t = p_c.tile([P, E], F32, name=nm)
                nc.sync.dma_start(out=t, in_=d[nm][:, :])
                gcons[nm] = t

        def load_wbf(nm):
            wt = p_w.tile([P, NF * E], BF16, tag=f"w{nm[1]}", name=nm)
            nc.sync.dma_start(
                out=wt, in_=d[nm][:, :].rearrange("(a p) e -> p a e", p=P))
            return wt

        def load_w8(nm, tag):
            wt = p_w8.tile([P, NF * E], FP8, tag=tag, name=nm)
            nc.sync.dma_start(out=wt, in_=d[nm][:, :])
            return wt

        def w8_rhs(wt, p):
            # rhs [128, 2, 512] for k-pair p
            return _ap3(wt, p * 2 * E, E, 2, 1, E)

        def w8_lhs(wt, p, m):
            # lhsT [128, 2, 128] for k-pair p, out col block m
            return _ap3(wt, p * 2 * E + m * P, E, 2, 1, P)

        def pair_lhs(pt_tile, p_idx, c):
            # activation pair tile [128, 2048]: k-tile i at cols i*1024
            return _ap3(pt_tile, c * P, S, 2, 1, P)

        def pair_rhs(pt_tile, nh):
            return _ap3(pt_tile, nh * 512, S, 2, 1, 512)

        def big_col(bt, k, c0, w):
            return bass.AP(tensor=bt.tensor, offset=bt.offset + k * S + c0,
                           ap=[list(bt.ap[0]), [1, w]])

        def bigT_ap(bt, c):
            # dma-transpose dest: [128, (S,4), (1,128)] at chunk col c
            return _ap3(bt, c * P, S, NF, 1, P)

        def grp(t, n=8, w=DH):
            return bass.AP(tensor=t.tensor, offset=t.offset,
                           ap=[list(t.ap[0]), [w, n], [1, w]])

        def bcast(t, n=8, w=DH):
            return bass.AP(tensor=t.tensor, offset=t.offset,
                           ap=[list(t.ap[0]), [1, n], [0, w]])

        # ---- input load + bf16 copy + dma-transpose ----
        def load_input(src_h, seq_tag, big_tag, pair_tag, q_alt):
            seqs = []
            big = p_big.tile([P, NF * S], BF16, tag=big_tag, name=big_tag)
            lds = []
            for qc in range(4):
                ld = p_ld.tile([P, 2 * E], F32, tag="ld", name=f"ld_{qc}",
                               bufs=2)
                nc.sync.dma_start(
                    out=ld, in_=src_h[qc * 256:(qc + 1) * 256, :]
                    .rearrange("(a p) e -> p a e", p=P))
                lds.append(ld)
            for c in range(NC):
                sq = p_seq.tile([P, E], BF16, tag=seq_tag, name=f"{seq_tag}{c}")
                eng = nc.vector if c % 2 else nc.gpsimd
                eng.tensor_copy(
                    sq, lds[c // 2][:, (c % 2) * E:(c % 2 + 1) * E])
                seqs.append(sq)
                qeng = nc.scalar if c % 2 else nc.sync
                qeng.dma_start_transpose(bigT_ap(big, c), sq)
            pairs = []
            for pi in range(2):
                pt = p_pair.tile([P, 2 * S], FP8, tag=f"{pair_tag}{pi}",
                                 name=f"{pair_tag}{pi}")
                for hf in range(2):
                    eng = nc.vector if (pi + hf) % 2 == 0 else nc.gpsimd
                    eng.tensor_copy(pt[:, hf * S:(hf + 1) * S],
                                    big[:, (pi * 2 + hf) * S:(pi * 2 + hf + 1) * S])
                pairs.append(pt)
            return seqs, big, pairs

        xb, xT, x8 = load_input(d["x"], "seqA", "bigA", "x8", 0)
        kqm_b = p_c.tile([P, NF * S], BF16, name="kqm_b")
        nc.sync.dma_start(
            out=kqm_b, in_=d["kqm"][:, :].rearrange("(a p) e -> p a e", p=P))
        kkm_b = p_c.tile([P, NF * S], BF16, name="kkm_b")
        nc.sync.dma_start(
            out=kkm_b, in_=d["kkm"][:, :].rearrange("(a p) e -> p a e", p=P))
        wq1t = load_wbf("wqc1"); wk1t = load_wbf("wkc1"); wv1t = load_wbf("wvc1")
        kp81 = load_w8("kp8_1", "kp8")
        ob, oT, o8 = load_input(d["obs"], "seqB", "bigB", "o8", 1)
        wg81 = load_w8("wg8_1", "wg8")
        wo81 = load_w8("wo8_1", "wo8")
        kksb = []
        for hc in range(2):
            kt = p_c.tile([P, 4 * E], BF16, name=f"kksb{hc}")
            nc.sync.dma_start(
                out=kt, in_=d["kks"][hc * 512:(hc + 1) * 512, :]
                .rearrange("(a p) e -> p a e", p=P))
            kksb.append(kt)
        wq2t = load_wbf("wqc2"); wk2t = load_wbf("wkc2"); wv2t = load_wbf("wvc2")
        kp82 = load_w8("kp8_2", "kp8"); wg82 = load_w8("wg8_2", "wg8")
        wo82 = load_w8("wo8_2", "wo8")
        fg8 = load_w8("fg8", "kp8"); fl8w = load_w8("fl8", "wg8")
        fo8 = load_w8("fo8", "wo8")

        def dump_seq(tiles, dt=F32):
            for c, t in enumerate(tiles[:NC]):
                o = p_ld.tile([P, E], F32, tag="dmp", name=f"dmp{c}")
                nc.vector.tensor_copy(o, t[:, :E])
                nc.sync.dma_start(out=out_h[c * P:(c + 1) * P, :], in_=o)

        def dump_fm(tiles, width=S):
            # feature-major tiles [128, width] -> out rows
            for k, t in enumerate(tiles[:NF]):
                o = p_ld.tile([P, S], F32, tag="dmpf", name=f"dmpf{k}", bufs=2)
                nc.vector.tensor_copy(o[:, :width], t[:, :width])
                rows = width // E
                for rr in range(rows):
                    nc.sync.dma_start(
                        out=out_h[(k * rows + rr) * P:(k * rows + rr + 1) * P, :],
                        in_=o[:, rr * E:(rr + 1) * E])

        # ================= MSR =================
        def msr(idx, qT_big, q8_pairs, kvT_big, kv8_pairs, resid_seq, wq_t, wk_t,
                wv_t, kp8, wg8, wo8, gn_triv, gsb, gbb, out_seq_tag, tap=""):
            sfx = str(idx)
            # --- q~ / k~ feature-major gemms (bf16) + decay-map evac ---
            qs = []
            ks = []
            for pt in range(NF):
                qs.append(p_act.tile([P, S], BF16, tag=f"qs{pt}", name=f"qs{sfx}_{pt}"))
                ks.append(p_act.tile([P, S], BF16, tag=f"ks{pt}", name=f"ks{sfx}_{pt}"))
            for pt in range(NF):
                for nh in range(2):
                    ps = pg.tile([P, 512], F32, tag="pg", name=f"q_{pt}_{nh}")
                    for k in range(NF):
                        nc.tensor.matmul(ps, wq_t[:, k * E + pt * P:
                                                   k * E + (pt + 1) * P],
                                         big_col(qT_big, k, nh * 512, 512),
                                         start=(k == 0), stop=(k == NF - 1))
                    nc.vector.tensor_mul(
                        qs[pt][:, nh * 512:(nh + 1) * 512], ps,
                        kqm_b[:, pt * S + nh * 512:pt * S + (nh + 1) * 512])
            for pt in range(NF):
                for nh in range(2):
                    ps = pg.tile([P, 512], F32, tag="pg", name=f"k_{pt}_{nh}")
                    for k in range(NF):
                        nc.tensor.matmul(ps, wk_t[:, k * E + pt * P:
                                                   k * E + (pt + 1) * P],
                                         big_col(kvT_big, k, nh * 512, 512),
                                         start=(k == 0), stop=(k == NF - 1))
                    nc.vector.tensor_mul(
                        ks[pt][:, nh * 512:(nh + 1) * 512], ps,
                        kkm_b[:, pt * S + nh * 512:pt * S + (nh + 1) * 512])
            if tap == "qs":
                dump_fm(qs)
                return None
            if tap == "ks":
                dump_fm(ks)
                return None

            # --- V (bf16) and K_seq (fp8 DR) seq-major gemms ---
            V = []
            Kq = []
            for c in range(NC):
                ps = pg.tile([P, 512], F32, tag="pg", name=f"v_{c}")
                for k in range(NF):
                    nc.tensor.matmul(ps, big_col(kvT_big, k, c * P, P),
                                     wv_t[:, k * E:(k + 1) * E],
                                     start=(k == 0), stop=(k == NF - 1))
                vt = p_kv.tile([P, 512], BF16, tag=f"V{c}", name=f"V{sfx}_{c}")
                nc.scalar.copy(vt, ps)
                V.append(vt)
            for c in range(NC):
                ps = pg.tile([P, 512], F32, tag="pg", name=f"kq_{c}")
                for pi in range(2):
                    nc.tensor.matmul(ps, pair_lhs(kv8_pairs[pi], pi, c),
                                     w8_rhs(kp8, pi), start=(pi == 0),
                                     stop=(pi == 1), perf_mode=DRM)
                kt = p_kv.tile([P, 512], BF16, tag=f"K{c}", name=f"K{sfx}_{c}")
                nc.vector.tensor_mul(
                    kt, ps, kksb[c // 4][:, (c % 4) * E:(c % 4 + 1) * E])
                Kq.append(kt)
            if tap == "V":
                dump_seq(V)
                return None
            if tap == "Kq":
                dump_seq(Kq)
                return None

            # --- gate gemm (fp8 DR, feature-major) + silu ---
            gfm = [p_act.tile([P, S], BF16, tag=f"gf{m}", name=f"gfm{sfx}_{m}")
                   for m in range(NF)]
            for m in ([] if "gate" in skips else range(NF)):
                for nh in range(2):
                    ps = pg.tile([P, 512], F32, tag="pg", name=f"g_{m}_{nh}")
                    for pi in range(2):
                        nc.tensor.matmul(ps, w8_lhs(wg8, pi, m),
                                         pair_rhs(q8_pairs[pi], nh),
                                         start=(pi == 0), stop=(pi == 1),
                                         perf_mode=DRM)
                    nc.scalar.activation(gfm[m][:, nh * 512:(nh + 1) * 512], ps,
                                         AF.Silu)

            # --- retention chunks (software-pipelined, batched GN stats) ---
            stc = []
            rn = []
            ret_sb = []
            sc_all = []
            sums_all = p_sm.tile([P, 64], F32, tag="sumsA", name=f"sumsA{sfx}",
                                 bufs=2)
            sqs_all = p_sm.tile([P, 64], F32, tag="sqsA", name=f"sqsA{sfx}",
                                bufs=2)

            def emit_scores(c):
                # bank parity = head parity so every matmul in a bank shares
                # the same tile_position row base (mixing row bases in one
                # bank breaks execution)
                sc_sb = []
                for par in range(2):
                    ps = psc.tile([P, 512], F32, tag="psc", name=f"sc_{c}_{par}")
                    sl = par * DH
                    for hh in range(4):
                        h = 2 * hh + par
                        ptt = h // 2
                        nc.tensor.matmul(
                            ps[:, hh * P:(hh + 1) * P],
                            ks[ptt][sl:sl + DH, c * P:(c + 1) * P],
                            qs[ptt][sl:sl + DH, c * P:(c + 1) * P],
                            start=(hh == 0), stop=(hh == 3),
                            skip_group_check=True)
                    sb = p_sc.tile([P, 512], BF16, tag=f"scsb{par}",
                                   name=f"scsb_{c}_{par}")
                    nc.vector.tensor_mul(sb, ps, cmask4)
                    sc_sb.append(sb)
                sc_all.append(sc_sb)

            def emit_stage(c):
                # state update first so chunk c+1's cross input is ready early
                if c < NC - 1 and "state" not in skips:
                    pstt = pst.tile([P, 256], F32, tag="pst", name=f"st_{c}",
                                    bufs=1)
                    for h in range(H):
                        ptt, sl = h // 2, (h % 2) * DH
                        nc.tensor.matmul(
                            pstt[sl:sl + DH, ptt * DH:(ptt + 1) * DH],
                            Kq[c][:, h * DH:(h + 1) * DH],
                            V[c][:, h * DH:(h + 1) * DH],
                            start=(h < 2), stop=(h >= H - 2),
                            skip_group_check=True, tile_position=(0, sl))
                    st = p_sm.tile([P, 256], BF16, tag="stc", name=f"stc_{c}",
                                   bufs=2)
                    if c == 0:
                        nc.vector.tensor_copy(st, pstt)
                    else:
                        nc.vector.tensor_add(st, pstt, stc[c - 1])
                    stc.append(st)
                prt = pret.tile([P, 512], F32, tag="pret", name=f"ret_{c}")
                cross = (c > 0) and ("state" not in skips) and \
                    ("intra" not in skips)
                for h in ([] if "intra" in skips else range(H)):
                    nc.tensor.matmul(
                        prt[:, h * DH:(h + 1) * DH],
                        sc_all[c][h % 2][:, (h // 2) * P:(h // 2 + 1) * P],
                        V[c][:, h * DH:(h + 1) * DH],
                        start=(h == 0), stop=(h == H - 1 and not cross),
                        skip_group_check=True)
                if cross:
                    for h in range(0, H, 2):       # even heads: row base 0
                        ptt = h // 2
                        nc.tensor.matmul(
                            prt[:, h * DH:(h + 1) * DH],
                            qs[ptt][0:DH, c * P:(c + 1) * P],
                            stc[c - 1][0:DH, ptt * DH:(ptt + 1) * DH],
                            start=False, stop=(h == H - 2),
                            skip_group_check=True)
                    pco = pcro.tile([P, 256], F32, tag="pcro", name=f"cro_{c}")
                    for h in range(1, H, 2):       # odd heads: row base 64
                        ptt = h // 2
                        nc.tensor.matmul(
                            pco[:, ptt * DH:(ptt + 1) * DH],
                            qs[ptt][DH:2 * DH, c * P:(c + 1) * P],
                            stc[c - 1][DH:2 * DH, ptt * DH:(ptt + 1) * DH],
                            start=(h == 1), stop=(h == H - 1),
                            skip_group_check=True)
                rb = p_seq.tile([P, 512], BF16, tag="retsb", name=f"retsb_{c}")
                if "intra" in skips:
                    nc.vector.memset(rb, 0.0)
                else:
                    nc.scalar.copy(rb, prt)
                    if cross:
                        def _odd(t, w):
                            return bass.AP(tensor=t.tensor,
                                           offset=t.offset + (DH if w else 0),
                                           ap=[list(t.ap[0]),
                                               [2 * DH if w else DH, 4],
                                               [1, DH]])
                        nc.vector.tensor_add(_odd(rb, True), _odd(rb, True),
                                             _odd(pco, False))
                ret_sb.append(rb)
                if "gn" not in skips:
                    sq = p_rot.tile([P, 512], BF16, tag="gnsq",
                                    name=f"gnsq_{c}")
                    nc.scalar.activation(sq, rb, AF.Square)
                    nc.vector.tensor_reduce(sums_all[:, c * H:(c + 1) * H],
                                            grp(rb),
                                            axis=mybir.AxisListType.X,
                                            op=ALU.add)
                    nc.vector.tensor_reduce(sqs_all[:, c * H:(c + 1) * H],
                                            grp(sq),
                                            axis=mybir.AxisListType.X,
                                            op=ALU.add)

            # narrow GN stats in two chunk-halves so the first half's
            # applies + rnT transposes overlap the second half's retention
            mu = p_sm.tile([P, 64], F32, tag="mu", name=f"mu{sfx}", bufs=1)
            m2 = p_sm.tile([P, 64], F32, tag="m2", name=f"m2{sfx}", bufs=1)
            msq = p_sm.tile([P, 64], F32, tag="msq", name=f"msq{sfx}", bufs=1)
            var = p_sm.tile([P, 64], F32, tag="var", name=f"var{sfx}", bufs=1)
            sd = p_sm.tile([P, 64], F32, tag="sd", name=f"sd{sfx}", bufs=1)
            rs = p_sm.tile([P, 64], F32, tag="rs", name=f"rs{sfx}", bufs=1)
            c2 = p_sm.tile([P, 64], F32, tag="c2", name=f"c2{sfx}", bufs=1)

            def bc8(t, c):
                return bass.AP(tensor=t.tensor, offset=t.offset + c * H,
                               ap=[list(t.ap[0]), [1, H], [0, DH]])

            def stats_and_apply(hf):
                hs = slice(hf * 32, hf * 32 + 32)
                nc.vector.tensor_scalar_mul(mu[:, hs], sums_all[:, hs],
                                            1.0 / DH)
                nc.gpsimd.tensor_mul(m2[:, hs], mu[:, hs], mu[:, hs])
                nc.gpsimd.tensor_scalar_mul(msq[:, hs], sqs_all[:, hs],
                                            1.0 / DH)
                nc.vector.tensor_sub(var[:, hs], msq[:, hs], m2[:, hs])
                nc.scalar.activation(sd[:, hs], var[:, hs], AF.Sqrt,
                                     bias=eps_gn)
                nc.vector.reciprocal(rs[:, hs], sd[:, hs])
                nc.gpsimd.tensor_mul(c2[:, hs], mu[:, hs], rs[:, hs])
                for c in range(hf * 4, hf * 4 + 4):
                    tmp = p_rot.tile([P, 512], BF16, tag="gntmp",
                                     name=f"gntmp_{c}")
                    nc.gpsimd.tensor_mul(grp(tmp), grp(ret_sb[c]), bc8(rs, c))
                    rt = p_rot.tile([P, 512], BF16, tag="rn", name=f"rn_{c}",
                                    bufs=2)
                    nc.gpsimd.tensor_sub(grp(rt), grp(tmp), bc8(c2, c))
                    if not gn_triv:
                        nc.vector.tensor_mul(rt, rt, gsb)
                        nc.vector.tensor_add(rt, rt, gbb)
                    rn.append(rt)
                    if tap != "rnnt":
                        qeng = nc.scalar if c % 2 else nc.sync
                        qeng.dma_start_transpose(bigT_ap(rnT, c), rt)

            import os as _os2
            if _os2.environ.get("KNOSPLIT"):
                emit_scores(0)
                for c in range(1, NC):
                    emit_scores(c)
                    emit_stage(c - 1)
                emit_stage(NC - 1)
                if tap == "ret":
                    dump_seq(ret_sb)
                    return None
                stats_and_apply(0)
                stats_and_apply(1)
            else:
                emit_scores(0)
                for c in range(1, NC):
                    emit_scores(c)
                    emit_stage(c - 1)
                    if c == 5:
                        stats_and_apply(0)
                emit_stage(NC - 1)
                if tap == "ret":
                    dump_seq(ret_sb)
                    return None
                stats_and_apply(1)
            if tap in ("rn", "rnnt"):
                dump_seq(rn)
                return None


            # --- gating (feature-major) -> fp8 pair tiles ---
            g8 = [p_pair.tile([P, 2 * S], FP8, tag=f"gated{pi}",
                              name=f"gated{sfx}_{pi}") for pi in range(2)]
            for m in range(NF):
                nc.gpsimd.tensor_mul(
                    g8[m // 2][:, (m % 2) * S:(m % 2 + 1) * S],
                    gfm[m], rnT[:, m * S:(m + 1) * S])

            # --- W_O gemm (fp8 DR, seq-major out) + residual + RMSNorm ---
            outs = []
            for c in range(NC):
                ps = pg.tile([P, 512], F32, tag="pg", name=f"wo_{c}")
                for pi in range(2):
                    nc.tensor.matmul(ps, pair_lhs(g8[pi], pi, c),
                                     w8_rhs(wo8, pi), start=(pi == 0),
                                     stop=(pi == 1), perf_mode=DRM)
                res = p_res.tile([P, E], F32, tag="res", name=f"res{sfx}_{c}")
                nc.vector.tensor_add(res, ps, resid_seq[c])
                ssq = p_sm.tile([P, 1], F32, tag="ssq", name=f"ssq_{c}", bufs=2)
                ts = p_rot.tile([P, E], BF16, tag="gnsq", name=f"ttr_{c}")
                nc.scalar.activation(ts, res, AF.Square)
                nc.vector.tensor_reduce(ssq, ts, axis=mybir.AxisListType.X,
                                        op=ALU.add)
                sdr = p_sm.tile([P, 1], F32, tag="sdr", name=f"sdr_{c}", bufs=2)
                nc.scalar.activation(sdr, ssq, AF.Sqrt, bias=eps_rms,
                                     scale=1.0 / E)
                rsr = p_sm.tile([P, 1], F32, tag="rsr", name=f"rsr_{c}", bufs=2)
                nc.vector.reciprocal(rsr, sdr)
                o = p_seq.tile([P, E], BF16, tag=out_seq_tag,
                               name=f"{out_seq_tag}{c}")
                nc.scalar.activation(o, res, AF.Identity, scale=rsr)
                outs.append(o)
            return outs

        # rnT big tile shared by both msrs (rotates)
        rnT = p_big.tile([P, NF * S], BF16, tag="bigR", name="rnT1")
        r = msr(1, xT, x8, xT, x8, xb, wq1t, wk1t,
                wv1t, kp81, wg81, wo81,
                gn1_triv, gcons.get("gsb1"), gcons.get("gbb1"), "seqA",
                tap=tap if tap in ("qs", "ks", "V", "Kq", "rn", "ret", "rnnt") else "")
        if tap in ("qs", "ks", "V", "Kq", "rn", "ret", "rnnt"):
            return
        x1 = r
        if tap == "x1":
            dump_seq(x1)
            return

        # x1 -> feature-major (dma transpose) + fp8 pairs
        x1T = p_big.tile([P, NF * S], BF16, tag="bigA", name="x1T")
        for c in range(NC):
            qeng = nc.scalar if c % 2 else nc.sync
            qeng.dma_start_transpose(bigT_ap(x1T, c), x1[c])
        x18 = []
        for pi in range(2):
            pt = p_pair.tile([P, 2 * S], FP8, tag=f"x8{pi}", name=f"x18_{pi}")
            for hf in range(2):
                eng = nc.vector if (pi + hf) % 2 == 0 else nc.gpsimd
                eng.tensor_copy(pt[:, hf * S:(hf + 1) * S],
                                x1T[:, (pi * 2 + hf) * S:(pi * 2 + hf + 1) * S])
            x18.append(pt)

        rnT = p_big.tile([P, NF * S], BF16, tag="bigR", name="rnT2")
        r = msr(2, oT, o8, x1T, x18, ob, wq2t, wk2t,
                wv2t, kp82, wg82, wo82,
                gn2_triv, gcons.get("gsb2"), gcons.get("gbb2"), "seqC")
        x2 = r
        if not ln2_triv:
            for c in range(NC):
                nc.gpsimd.tensor_mul(x2[c], x2[c], gcons["ln2C"])
        if tap == "x2":
            dump_seq(x2)
            return

        # x2 -> feature-major + fp8 pairs (ffn inputs)
        x2T = p_big.tile([P, NF * S], BF16, tag="bigR", name="x2T")
        for c in range(NC):
            qeng = nc.scalar if c % 2 else nc.sync
            qeng.dma_start_transpose(bigT_ap(x2T, c), x2[c])
        x28 = []
        for pi in range(2):
            pt = p_pair.tile([P, 2 * S], FP8, tag=f"o8{pi}", name=f"x28_{pi}")
            for hf in range(2):
                eng = nc.vector if (pi + hf) % 2 == 0 else nc.gpsimd
                eng.tensor_copy(pt[:, hf * S:(hf + 1) * S],
                                x2T[:, (pi * 2 + hf) * S:(pi * 2 + hf + 1) * S])
            x28.append(pt)

        # ---- FFN (all fp8 DR) ----
        ffg = [p_act.tile([P, S], BF16, tag=f"qs{m}", name=f"ffg_{m}")
               for m in range(NF)]
        ffl = [p_act.tile([P, S], BF16, tag=f"ks{m}", name=f"ffl_{m}")
               for m in range(NF)]
        for m in range(NF):
            for nh in range(2):
                ps = pg.tile([P, 512], F32, tag="pg", name=f"fg_{m}_{nh}")
                for pi in range(2):
                    nc.tensor.matmul(ps, w8_lhs(fg8, pi, m),
                                     pair_rhs(x28[pi], nh), start=(pi == 0),
                                     stop=(pi == 1), perf_mode=DRM)
                nc.scalar.activation(ffg[m][:, nh * 512:(nh + 1) * 512], ps,
                                     AF.Silu)
        for m in range(NF):
            for nh in range(2):
                ps = pg.tile([P, 512], F32, tag="pg", name=f"fl_{m}_{nh}")
                for pi in range(2):
                    nc.tensor.matmul(ps, w8_lhs(fl8w, pi, m),
                                     pair_rhs(x28[pi], nh), start=(pi == 0),
                                     stop=(pi == 1), perf_mode=DRM)
                nc.vector.tensor_copy(ffl[m][:, nh * 512:(nh + 1) * 512], ps)
        fl8t = [p_pair.tile([P, 2 * S], FP8, tag=f"gated{pi}", name=f"flT8_{pi}")
                for pi in range(2)]
        for m in range(NF):
            nc.gpsimd.tensor_mul(fl8t[m // 2][:, (m % 2) * S:(m % 2 + 1) * S],
                                 ffg[m], ffl[m])
        for c in range(NC):
            ps = pg.tile([P, 512], F32, tag="pg", name=f"fo_{c}")
            for pi in range(2):
                nc.tensor.matmul(ps, pair_lhs(fl8t[pi], pi, c),
                                 w8_rhs(fo8, pi), start=(pi == 0),
                                 stop=(pi == 1), perf_mode=DRM)
            res = p_res.tile([P, E], F32, tag="res", name=f"res3_{c}")
            nc.vector.tensor_add(res, ps, x2[c])
            ssq = p_sm.tile([P, 1], F32, tag="ssq", name=f"ssq3_{c}", bufs=2)
            ts = p_rot.tile([P, E], BF16, tag="gnsq", name=f"ttr3_{c}")
            nc.scalar.activation(ts, res, AF.Square)
            nc.vector.tensor_reduce(ssq, ts, axis=mybir.AxisListType.X,
                                    op=ALU.add)
            sdr = p_sm.tile([P, 1], F32, tag="sdr", name=f"sdr3_{c}", bufs=2)
            nc.scalar.activation(sdr, ssq, AF.Sqrt, bias=eps_rms, scale=1.0 / E)
            rsr = p_sm.tile([P, 1], F32, tag="rsr", name=f"rsr3_{c}", bufs=2)
            nc.vector.reciprocal(rsr, sdr)
            if c % 4 == 0:
                obig = p_ld.tile([P, 4 * E], F32, tag="oo", name=f"oo_{c // 4}",
                                 bufs=1)
            o = obig[:, (c % 4) * E:(c % 4 + 1) * E]
            nc.scalar.activation(o, res, AF.Identity, scale=rsr)
            if not ln3_triv:
                nc.gpsimd.tensor_mul(o, o, gcons["ln3C"])
            if c % 4 == 3:
                nc.sync.dma_start(
                    out=out_h[(c - 3) * P:(c + 1) * P, :]
                    .rearrange("(a p) e -> p a e", p=P), in_=obig)


_prog_cache = {}


def _get_program(flags=(True, True, True, True)):
    if flags not in _prog_cache:
        _prog_cache[flags] = _build_program(flags)
    return _prog_cache[flags]


def kernel(**inputs):
    inputs = {k: np.asarray(v) for k, v in inputs.items()}
    flags = _flags(inputs)
    consts = _build_consts(inputs)
    pr = _get_program(flags)
    x = np.ascontiguousarray(inputs["x"], dtype=np.float32)
    obs = np.ascontiguousarray(inputs["obs_rep"], dtype=np.float32)
    in_maps = []
    for b in range(N_CORES):
        m = dict(consts)
        m["x"] = np.ascontiguousarray(x[b])
        m["obs"] = np.ascontiguousarray(obs[b])
        in_maps.append(m)
    res = run_bass_kernel_spmd(pr.nc, in_maps, core_ids=list(range(N_CORES)))
    return np.stack([res.results[b]["out"] for b in range(N_CORES)], axis=0)

